# revision 26
# baseline (speedup 1.0000x reference)
"""Trainium2 Bass kernel for nn_Decoder (2-layer LSTM decoder + Luong attention + vocab projection).

Strategy (8 NeuronCores):
  - The LSTM recurrence (the only truly sequential part: attention context does
    NOT feed back into the carry) is replicated on every core at full batch
    B=32 -- its cost is weight-streaming bound and independent of batch, so
    replication is free and avoids per-step collectives.
  - Attention (energy/softmax/context) is batched over timesteps and
    batch-sharded: each core handles 4 batches (its encoder slice), computed in
    T-chunks pipelined with the recurrence; contexts are AllGather'd.
  - The dominant [T*B, 2H] @ [2H, V] logit projection is vocab-sharded: each
    core computes its V/8 slice in bf16 and the host concatenates slices.
  - All matmuls run in bf16 (weights cast+transposed on device via DMA-xbar
    transpose, off the PE critical path); cell state c stays f32.
"""

import os
import sys

for _p in ("/opt/trn_rl_repo",):
    if _p not in sys.path and os.path.isdir(_p):
        sys.path.insert(0, _p)

import numpy as np

import concourse.bass as bass
import concourse.mybir as mybir
import concourse.tile as tile
from concourse import bacc
from concourse.bass_utils import run_bass_kernel_spmd

dt = mybir.dt
ALU = mybir.AluOpType
ACTF = mybir.ActivationFunctionType

N_CORES = 8
B, S, H, E, V = 32, 512, 512, 300, 32000
NEG = -1.0e12

LAST_RESULT = None  # BassKernelResults of the most recent run (for test harness)
_BUILD_CACHE = {}
DEBUG_ATT = False  # dump attention intermediates of chunk 0 to DRAM


def build_decoder_nc(T):
    """Build the SPMD Bass program for sequence length T (B=32 fixed)."""
    assert T % 4 == 0
    TB = T * B                      # q rows, ordered row = 32*t + b
    ATT = 32 if T % 32 == 0 else T  # attention T-chunk
    NCH = T // ATT                  # number of attention chunks
    RM = TB // 128                  # logit row chunks (4 steps each)
    Vc = V // N_CORES               # vocab slice per core
    KE = [128, 128, 44]             # K-chunks over E=300
    XC = min(512, TB)               # X0 matmul N-chunk (cols of (t,b))
    NRC = TB // XC
    # vocab sub-chunks: multiples of 16 (xbar), <=512 (psum bank)
    vchunks = []
    v0 = 0
    while v0 < Vc:
        vchunks.append((v0, min(512, Vc - v0)))
        v0 += 512

    nc = bacc.Bacc("TRN2", target_bir_lowering=False, debug=False,
                   num_devices=N_CORES, enable_partition_id=True)

    # ---------------- I/O ----------------
    trg = nc.dram_tensor("trg_seq", [B, T], dt.int32, kind="ExternalInput")
    h0T_in = nc.dram_tensor("h0T", [2, H, B], dt.float32, kind="ExternalInput")
    c0T_in = nc.dram_tensor("c0T", [2, H, B], dt.float32, kind="ExternalInput")
    enc_in = nc.dram_tensor("enc_loc", [4, S, H], dt.float32, kind="ExternalInput")
    mask_in = nc.dram_tensor("mask_loc", [4, S], dt.uint8, kind="ExternalInput")
    embed_in = nc.dram_tensor("embed", [V, E], dt.float32, kind="ExternalInput")
    wenc_in = nc.dram_tensor("W_enc", [H, H], dt.float32, kind="ExternalInput")
    benc_in = nc.dram_tensor("b_enc", [H], dt.float32, kind="ExternalInput")
    wih0_in = nc.dram_tensor("W_ih0", [4 * H, E], dt.float32, kind="ExternalInput")
    whh0_in = nc.dram_tensor("W_hh0", [4 * H, H], dt.float32, kind="ExternalInput")
    b0_in = nc.dram_tensor("b0", [4 * H], dt.float32, kind="ExternalInput")
    wih1_in = nc.dram_tensor("W_ih1", [4 * H, H], dt.float32, kind="ExternalInput")
    whh1_in = nc.dram_tensor("W_hh1", [4 * H, H], dt.float32, kind="ExternalInput")
    b1_in = nc.dram_tensor("b1", [4 * H], dt.float32, kind="ExternalInput")
    wlog_in = nc.dram_tensor("W_logit_loc", [Vc, 2 * H], dt.float32,
                             kind="ExternalInput")
    blog_in = nc.dram_tensor("b_logit_loc", [Vc], dt.float32, kind="ExternalInput")
    out_t = nc.dram_tensor("out", [B, T, Vc], dt.float32, kind="ExternalOutput")

    with tile.TileContext(nc) as tc:
        with (
            tc.tile_pool(name="const", bufs=1) as cp,
            tc.tile_pool(name="work", bufs=2) as wp,
            tc.tile_pool(name="psum", bufs=6, space="PSUM") as pp,
            tc.tile_pool(name="dram", bufs=1, space="DRAM") as dp,
        ):
            # ---------- helpers ----------
            def cast_to_scratch(src_ap, rows, cols, scr, col_off=0):
                """f32 DRAM [rows, cols] -> bf16 into scr[:, col_off:col_off+cols],
                in [128, <=512] stages."""
                for m in range((rows + 127) // 128):
                    h = min(128, rows - 128 * m)
                    for c0 in range(0, cols, 512):
                        cw = min(512, cols - c0)
                        sf = wp.tile([128, 512], dt.float32, tag="ct_f32")
                        nc.sync.dma_start(
                            sf[0:h, 0:cw],
                            src_ap[128 * m:128 * m + h, c0:c0 + cw])
                        sb = wp.tile([128, 512], dt.bfloat16, tag="ct_bf")
                        nc.vector.tensor_copy(sb[0:h, 0:cw], sf[0:h, 0:cw])
                        nc.sync.dma_start(
                            scr[128 * m:128 * m + h,
                                col_off + c0:col_off + c0 + cw],
                            sb[0:h, 0:cw])

            def transpose_in(t_sb_ap, scr, col0):
                """scr[:, col0:col0+128] (bf16 DRAM) -> t_sb_ap [128, rows]."""
                nc.sync.dma_start_transpose(t_sb_ap, scr[:, col0:col0 + 128])

            def load_vec_bf16(pool, vec_dram, n, name):
                """f32 DRAM [n] -> bf16 SBUF [1, n] via a packed 2D stage."""
                p = 128 if n % 128 == 0 else 125
                assert n % p == 0
                m = n // p
                vf = wp.tile([128, 32], dt.float32, tag="vec_f32")
                nc.sync.dma_start(
                    vf[0:p, 0:m], vec_dram[:].rearrange("(p m) -> p m", p=p))
                vb = wp.tile([128, 32], dt.bfloat16, tag="vec_bf")
                nc.vector.tensor_copy(vb[0:p, 0:m], vf[0:p, 0:m])
                out = pool.tile([1, n], dt.bfloat16, tag=name, name=name)
                nc.sync.dma_start(
                    out[:, :].rearrange("o (pp m) -> o pp m", pp=p),
                    vb[0:p, 0:m])
                return out

            # ---------- persistent constants ----------
            pid = nc.vector.partition_id()
            boff = pid * 4  # first local batch (global index) of this core
            ones = cp.tile([1, 512], dt.bfloat16, tag="ones")
            nc.vector.memset(ones[:, :], 1.0)
            blogsb = load_vec_bf16(cp, blog_in, Vc, "blogsb")
            mask_u8 = wp.tile([1, 4, S], dt.uint8, tag="mask_u8")
            nc.sync.dma_start(mask_u8[0:1, :, :], mask_in[:])
            maskneg = cp.tile([1, 4, S], dt.bfloat16, tag="maskneg")
            nc.vector.tensor_scalar(maskneg[:, :, :], mask_u8[:, :, :],
                                    float(NEG), None, ALU.mult)

            def load_state(src, layer, dtype, tag):
                t = cp.tile([128, 4, B], dtype, tag=tag, name=tag)
                if dtype == dt.float32:
                    nc.sync.dma_start(
                        t[:, :, :],
                        src[layer].rearrange("(kc p) b -> p kc b", p=128))
                else:
                    f = wp.tile([128, 4, B], dt.float32, tag="st_f")
                    nc.sync.dma_start(
                        f[:, :, :],
                        src[layer].rearrange("(kc p) b -> p kc b", p=128))
                    nc.vector.tensor_copy(t[:, :, :], f[:, :, :])
                return t

            h0T_init = load_state(h0T_in, 0, dt.bfloat16, "h0T_init")
            h1T_init = load_state(h0T_in, 1, dt.bfloat16, "h1T_init")
            c0T_init = load_state(c0T_in, 0, dt.float32, "c0T_init")
            c1T_init = load_state(c0T_in, 1, dt.float32, "c1T_init")

            # ---------- recurrence weights (persistent SBUF) ----------
            scr_cat = dp.tile([4 * H, 1152], dt.bfloat16, tag="scr_cat")
            cast_to_scratch(wih1_in[:], 4 * H, H, scr_cat, 0)
            cast_to_scratch(whh1_in[:], 4 * H, H, scr_cat, H)
            b1_ld = wp.tile([128, 16], dt.float32, tag="vec_f32")
            nc.sync.dma_start(b1_ld[:, :],
                              b1_in[:].rearrange("(m p) -> p m", p=128))
            b1_bf = wp.tile([128, 16], dt.bfloat16, tag="vec_bf")
            nc.vector.tensor_copy(b1_bf[:, :], b1_ld[:, :])
            for m in range(16):
                nc.sync.dma_start(
                    scr_cat[128 * m:128 * (m + 1), 2 * H:2 * H + 1],
                    b1_bf[:, m:m + 1])
            w1T = cp.tile([128, 9, 4 * H], dt.bfloat16, tag="w1T")
            for kc in range(9):
                transpose_in(w1T[:, kc, :], scr_cat, 128 * kc)

            scr_hh0 = dp.tile([4 * H, 512], dt.bfloat16, tag="scr_hh0")
            cast_to_scratch(whh0_in[:], 4 * H, H, scr_hh0, 0)
            whh0T = cp.tile([128, 4, 4 * H], dt.bfloat16, tag="whh0T")
            for kc in range(4):
                transpose_in(whh0T[:, kc, :], scr_hh0, 128 * kc)

            # W_logit: cast to bf16 scratch only (transposed slices streamed
            # on the fly during the logits phase)
            scr_wlog = dp.tile([Vc, 2 * H], dt.bfloat16, tag="scr_wlog")
            cast_to_scratch(wlog_in[:], Vc, 2 * H, scr_wlog, 0)

            # ---------- memories -> DRAM (both layouts, bf16) ----------
            memhs_dram = dp.tile([4, 4, 128, S], dt.bfloat16, tag="memhs_dram")
            memsh_dram = dp.tile([4, 4, 128, H], dt.bfloat16, tag="memsh_dram")
            with tc.tile_pool(name="prep", bufs=1) as pm:
                px = pm
                scr_enc = dp.tile([4 * S, H], dt.bfloat16, tag="scr_enc")
                cast_to_scratch(
                    enc_in[:].rearrange("b s h -> (b s) h"), 4 * S, H,
                    scr_enc, 0)
                encT = pm.tile([128, 16, S], dt.bfloat16, tag="encT")
                for b in range(4):
                    for kc in range(4):
                        nc.sync.dma_start_transpose(
                            encT[:, 4 * b + kc, :],
                            scr_enc[S * b:S * (b + 1),
                                    128 * kc:128 * (kc + 1)])
                scr_wenc = dp.tile([H, 512], dt.bfloat16, tag="scr_wenc")
                cast_to_scratch(wenc_in[:], H, H, scr_wenc, 0)
                wencT = pm.tile([128, 4, H], dt.bfloat16, tag="wencT")
                for kc in range(4):
                    transpose_in(wencT[:, kc, :], scr_wenc, 128 * kc)
                bencsb = load_vec_bf16(pm, benc_in, H, "bencsb")

                for b in range(4):
                    for hm in range(4):
                        ps = pp.tile([128, S], dt.float32, tag="ps")
                        for kc in range(4):
                            nc.tensor.matmul(
                                ps[:, :],
                                wencT[:, kc, 128 * hm:128 * (hm + 1)],
                                encT[:, 4 * b + kc, :],
                                start=(kc == 0), stop=False)
                        nc.tensor.matmul(
                            ps[:, :], bencsb[0:1, 128 * hm:128 * (hm + 1)],
                            ones[0:1, 0:S], start=False, stop=True)
                        msb = wp.tile([128, S], dt.bfloat16, tag="msb")
                        nc.vector.tensor_copy(msb[:, :], ps[:, :])
                        nc.sync.dma_start(memhs_dram[b, hm, :, :], msb[:, :])
                    for sm in range(4):
                        ps = pp.tile([128, H], dt.float32, tag="ps")
                        for kc in range(4):
                            nc.tensor.matmul(
                                ps[:, :],
                                encT[:, 4 * b + kc, 128 * sm:128 * (sm + 1)],
                                wencT[:, kc, :],
                                start=(kc == 0), stop=False)
                        nc.tensor.matmul(
                            ps[:, :], ones[0:1, 0:128], bencsb[0:1, :],
                            start=False, stop=True)
                        msb = wp.tile([128, H], dt.bfloat16, tag="msb")
                        nc.vector.tensor_copy(msb[:, :], ps[:, :])
                        nc.sync.dma_start(memsh_dram[b, sm, :, :], msb[:, :])

            # ---------- embedding gather + X0 = xs @ W_ih0.T + b0 ----------
                x0_dram = dp.tile([T, 16, 128, B], dt.float32, tag="x0_dram")
                # xs rows are b-major (r = b*T + t) so the index load is a
                # plain flat reshape of trg_seq
                idx = px.tile([128, TB // 128], dt.int32, tag="idx")
                nc.sync.dma_start(
                    idx[:, :],
                    trg[:].rearrange("b t -> (b t)")
                    .rearrange("(m p) -> p m", p=128))
                scr_xs = dp.tile([TB, 384], dt.bfloat16, tag="scr_xs")
                for m in range(TB // 128):
                    xg = wp.tile([128, E], dt.float32, tag="xg")
                    nc.gpsimd.indirect_dma_start(
                        out=xg[:, :], out_offset=None, in_=embed_in[:],
                        in_offset=bass.IndirectOffsetOnAxis(
                            ap=idx[:, m:m + 1], axis=0))
                    xb = wp.tile([128, E], dt.bfloat16, tag="xb")
                    nc.vector.tensor_copy(xb[:, :], xg[:, :])
                    nc.sync.dma_start(scr_xs[128 * m:128 * (m + 1), 0:E],
                                      xb[:, :])
                xsT = px.tile([128, 3, TB], dt.bfloat16, tag="xsT")
                for kc in range(3):
                    transpose_in(xsT[:, kc, :], scr_xs, 128 * kc)

                scr_ih0 = dp.tile([4 * H, 384], dt.bfloat16, tag="scr_ih0")
                cast_to_scratch(wih0_in[:], 4 * H, E, scr_ih0, 0)
                wih0T = px.tile([128, 3, 4 * H], dt.bfloat16, tag="wih0T")
                for kc in range(3):
                    transpose_in(wih0T[:, kc, :], scr_ih0, 128 * kc)
                b0sb = load_vec_bf16(px, b0_in, 4 * H, "b0sb")

                for rc in range(NRC):
                    for gm in range(16):
                        ps = pp.tile([128, XC], dt.float32, tag="ps")
                        for i, ke in enumerate(KE):
                            nc.tensor.matmul(
                                ps[:, :],
                                wih0T[0:ke, i, 128 * gm:128 * (gm + 1)],
                                xsT[0:ke, i, XC * rc:XC * (rc + 1)],
                                start=(i == 0), stop=False)
                        nc.tensor.matmul(
                            ps[:, :], b0sb[0:1, 128 * gm:128 * (gm + 1)],
                            ones[0:1, 0:XC], start=False, stop=True)
                        sb = wp.tile([128, XC], dt.float32, tag="x0_sb")
                        nc.vector.tensor_copy(sb[:, :], ps[:, :])
                        b0c = XC * rc // T
                        nc.sync.dma_start(
                            x0_dram[:, gm, :, b0c:b0c + XC // T]
                            .rearrange("t p b -> p b t"),
                            sb[:, :])

            # =================================================================
            # Recurrence + attention chunks + ctx AllGather
            # =================================================================
            q_sb = cp.tile([128, 4, ATT, B], dt.bfloat16, tag="q_sb")
            h1carry = cp.tile([128, 4, B], dt.bfloat16, tag="h1carry")
            # per-chunk q in DRAM, layout [kc, p, t_local, b]
            q_drams = [dp.tile([8, 128, ATT, B], dt.bfloat16, tag=f"qd{k}",
                               name=f"qd{k}") for k in range(NCH)]
            cc_ins = [dp.tile([128, 4, ATT, 4], dt.bfloat16, tag=f"cci{k}",
                              name=f"cci{k}") for k in range(NCH)]
            cc_outs = [dp.tile([N_CORES, 128, 4, ATT, 4], dt.bfloat16,
                               tag=f"cco{k}", name=f"cco{k}")
                       for k in range(NCH)]

            h0T, c0T, c1T = h0T_init, c0T_init, c1T_init

            def nonlin(gs, cT, tag, h_out_ap):
                sgi = wp.tile([128, 4, B], dt.float32, tag=f"sgi{tag}")
                sgf = wp.tile([128, 4, B], dt.float32, tag=f"sgf{tag}")
                thg = wp.tile([128, 4, B], dt.float32, tag=f"thg{tag}")
                sgo = wp.tile([128, 4, B], dt.float32, tag=f"sgo{tag}")
                nc.scalar.activation(sgi[:, :, :], gs[:, 0:4, :], ACTF.Sigmoid)
                nc.scalar.activation(sgf[:, :, :], gs[:, 4:8, :], ACTF.Sigmoid)
                nc.scalar.activation(thg[:, :, :], gs[:, 8:12, :], ACTF.Tanh)
                nc.scalar.activation(sgo[:, :, :], gs[:, 12:16, :], ACTF.Sigmoid)
                t1 = wp.tile([128, 4, B], dt.float32, tag=f"t1{tag}")
                nc.vector.tensor_tensor(t1[:, :, :], sgf[:, :, :],
                                        cT[:, :, :], ALU.mult)
                t2 = wp.tile([128, 4, B], dt.float32, tag=f"t2{tag}")
                nc.vector.tensor_tensor(t2[:, :, :], sgi[:, :, :],
                                        thg[:, :, :], ALU.mult)
                cn = wp.tile([128, 4, B], dt.float32, tag=f"cn{tag}", bufs=3)
                nc.vector.tensor_tensor(cn[:, :, :], t1[:, :, :],
                                        t2[:, :, :], ALU.add)
                thc = wp.tile([128, 4, B], dt.float32, tag=f"thc{tag}")
                nc.scalar.activation(thc[:, :, :], cn[:, :, :], ACTF.Tanh)
                nc.vector.tensor_tensor(h_out_ap, sgo[:, :, :],
                                        thc[:, :, :], ALU.mult)
                return cn

            def attention(k):
                """Attention for t-chunk k over the 4 local batches."""
                t0 = ATT * k
                # save h1 carry for the next chunk before q_sb is reused
                nc.vector.tensor_copy(h1carry[:, :, :],
                                      q_sb[:, :, ATT - 1, :])
                # store the (replicated) h1 part of q to DRAM for logits
                for kc in range(4):
                    nc.sync.dma_start(q_drams[k][kc, :, :, :],
                                      q_sb[:, kc, :, :])
                # local-batch h1 columns (core-dependent -> dynamic offset)
                hloc = cp.tile([128, 4, ATT, 4], dt.bfloat16, tag="hloc")
                nc.vector.tensor_copy(hloc[:, :, :, :],
                                      q_sb[:, :, :, bass.ds(boff, 4)])
                eps = pp.tile([128, S], dt.float32, tag="ps")
                if ATT < 32:
                    # test-path (T<32): zero the partitions the matmuls skip
                    nc.vector.memset(eps[:, :], 0.0)
                for b in range(4):
                    mh = wp.tile([128, 4, S], dt.bfloat16, tag="mh")
                    nc.sync.dma_start(mh[:, :, :],
                                      memhs_dram[b].rearrange("kc p s -> p kc s"))
                    for kc in range(4):
                        nc.tensor.matmul(
                            eps[32 * b:32 * b + ATT, :],
                            hloc[:, kc, :, b], mh[:, kc, :],
                            start=(kc == 0), stop=False,
                            skip_group_check=True, tile_position=(0, 32 * b))
                    nc.tensor.matmul(
                        eps[32 * b:32 * b + ATT, :], ones[0:1, 0:ATT],
                        maskneg[0:1, b, :], start=False, stop=True,
                        skip_group_check=True, tile_position=(0, 32 * b))
                mx = wp.tile([128, 1], dt.float32, tag="mx")
                nc.vector.reduce_max(mx[:, :], eps[:, :], mybir.AxisListType.X)
                e2 = wp.tile([128, S], dt.float32, tag="e2")
                nc.vector.tensor_scalar(e2[:, :], eps[:, :], mx[:, 0:1], None,
                                        ALU.subtract)
                pexp = wp.tile([128, S], dt.bfloat16, tag="pexp")
                sume = wp.tile([128, 1], dt.float32, tag="sume")
                nc.scalar.activation(pexp[:, :], e2[:, :], ACTF.Exp,
                                     accum_out=sume[:, :])
                rinv = wp.tile([128, 1], dt.float32, tag="rinv")
                nc.vector.reciprocal(rinv[:, :], sume[:, :])
                nc.vector.tensor_scalar(pexp[:, :], pexp[:, :], rinv[:, 0:1],
                                        None, ALU.mult)
                pT = wp.tile([128, 4, 128], dt.bfloat16, tag="pT")
                for ks in range(4):
                    nc.sync.dma_start_transpose(
                        pT[:, ks, :], pexp[:, 128 * ks:128 * (ks + 1)])
                if DEBUG_ATT and k == 0:
                    d_eps = dp.tile([128, S], dt.float32, tag="d_eps")
                    nc.sync.dma_start(d_eps[:, :], e2[:, :])  # eps - mx
                    d_pexp = dp.tile([128, S], dt.bfloat16, tag="d_pexp")
                    nc.sync.dma_start(d_pexp[:, :], pexp[:, :])
                    d_pt = dp.tile([128, 4, 128], dt.bfloat16, tag="d_pt")
                    nc.sync.dma_start(d_pt[:, :, :], pT[:, :, :])
                    d_mx = dp.tile([128, 2], dt.float32, tag="d_mx")
                    nc.sync.dma_start(d_mx[:, 0:1], mx[:, :])
                    nc.sync.dma_start(d_mx[:, 1:2], sume[:, :])
                ctxl = wp.tile([128, 4, ATT, 4], dt.bfloat16, tag="ctxl")
                for b in range(4):
                    msh = wp.tile([128, 4, H], dt.bfloat16, tag="msh")
                    nc.sync.dma_start(msh[:, :, :],
                                      memsh_dram[b].rearrange("sm p h -> p sm h"))
                    cps = pp.tile([128, 4, ATT], dt.float32, tag="ps")
                    for hm in range(4):
                        for ks in range(4):
                            nc.tensor.matmul(
                                cps[:, hm, :],
                                msh[:, ks, 128 * hm:128 * (hm + 1)],
                                pT[:, ks, 32 * b:32 * b + ATT],
                                start=(ks == 0), stop=(ks == 3))
                    nc.vector.tensor_copy(ctxl[:, :, :, b], cps[:, :, :])
                nc.sync.dma_start(cc_ins[k][:, :, :, :], ctxl[:, :, :, :])
                nc.gpsimd.collective_compute(
                    "AllGather", ALU.bypass,
                    replica_groups=[list(range(N_CORES))],
                    ins=[cc_ins[k].opt()], outs=[cc_outs[k].opt()])
                for hm in range(4):
                    for c in range(N_CORES):
                        nc.sync.dma_start(
                            q_drams[k][4 + hm, :, :, 4 * c:4 * (c + 1)],
                            cc_outs[k][c, :, hm, :, :])

            for t in range(T):
                k, dtc = t // ATT, t % ATT
                x0s = wp.tile([128, 16, B], dt.float32, tag="x0s", bufs=3)
                nc.sync.dma_start(x0s[:, :, :],
                                  x0_dram[t].rearrange("g p b -> p g b"))
                # ---- layer 0 ----
                g0 = pp.tile([128, 16, B], dt.float32, tag="ps")
                for gm in range(16):
                    for kc in range(4):
                        nc.tensor.matmul(
                            g0[:, gm, :],
                            whh0T[:, kc, 128 * gm:128 * (gm + 1)],
                            h0T[:, kc, :], start=(kc == 0), stop=(kc == 3))
                g0s = wp.tile([128, 16, B], dt.float32, tag="g0s")
                nc.vector.tensor_tensor(g0s[:, :, :], g0[:, :, :],
                                        x0s[:, :, :], ALU.add)
                h0Tn = wp.tile([128, 4, B], dt.bfloat16, tag="h0Tn", bufs=3)
                c0T = nonlin(g0s, c0T, "a", h0Tn[:, :, :])
                h0T = h0Tn
                # ---- layer 1 ----
                g1 = pp.tile([128, 16, B], dt.float32, tag="ps")
                for gm in range(16):
                    gsl = slice(128 * gm, 128 * (gm + 1))
                    for kc in range(4, 8):  # h1 half first (already available)
                        if t == 0:
                            rhs = h1T_init[:, kc - 4, :]
                        elif dtc == 0:
                            rhs = h1carry[:, kc - 4, :]
                        else:
                            rhs = q_sb[:, kc - 4, dtc - 1, :]
                        nc.tensor.matmul(g1[:, gm, :], w1T[:, kc, gsl], rhs,
                                         start=(kc == 4), stop=False)
                        # note: reading q_sb while also writing other cols is
                        # fine -- Tile serializes as needed
                    for kc in range(4):  # h0n half
                        nc.tensor.matmul(g1[:, gm, :], w1T[:, kc, gsl],
                                         h0T[:, kc, :], start=False, stop=False)
                    nc.tensor.matmul(g1[:, gm, :], w1T[0:1, 8, gsl],
                                     ones[0:1, 0:B], start=False, stop=True)
                c1T = nonlin(g1, c1T, "b", q_sb[:, :, dtc, :])
                if dtc == ATT - 1:
                    attention(k)

            # =================================================================
            # Logits: out = q @ W_logit_loc.T + b_logit  (vocab-sliced)
            # =================================================================
            for (vlo, vw) in vchunks:
                wlt = wp.tile([128, 8, 512], dt.bfloat16, tag="wlt", bufs=2)
                for kc in range(8):
                    nc.sync.dma_start_transpose(
                        wlt[:, kc, 0:vw],
                        scr_wlog[vlo:vlo + vw, 128 * kc:128 * (kc + 1)])
                for rm in range(RM):
                    k = (4 * rm) // ATT
                    tloc = (4 * rm) % ATT
                    qrm = wp.tile([128, 8, 4, B], dt.bfloat16, tag="qrm",
                                  bufs=3)
                    nc.sync.dma_start(
                        qrm[:, :, :, :],
                        q_drams[k][:, :, tloc:tloc + 4, :]
                        .rearrange("kc p t b -> p kc t b"))
                    ps = pp.tile([128, 512], dt.float32, tag="ps")
                    for kc in range(8):
                        nc.tensor.matmul(
                            ps[:, 0:vw], qrm[:, kc, :, :],
                            wlt[:, kc, 0:vw],
                            start=(kc == 0), stop=False)
                    nc.tensor.matmul(
                        ps[:, 0:vw], ones[0:1, 0:128],
                        blogsb[0:1, vlo:vlo + vw], start=False, stop=True)
                    osb = wp.tile([128, 512], dt.float32, tag="osb", bufs=3)
                    if rm % 2 == 0:
                        nc.vector.tensor_copy(osb[:, 0:vw], ps[:, 0:vw])
                    else:
                        nc.scalar.copy(osb[:, 0:vw], ps[:, 0:vw])
                    nc.sync.dma_start(
                        out_t[:, 4 * rm:4 * rm + 4, vlo:vlo + vw]
                        .rearrange("b t v -> t b v"),
                        osb[:, 0:vw])

    nc.compile()
    return nc


def kernel(**inputs):
    global LAST_RESULT
    trg_seq = np.ascontiguousarray(np.asarray(inputs["trg_seq"], dtype=np.int32))
    B_, T = trg_seq.shape
    assert B_ == B
    h0 = np.asarray(inputs["h0"], dtype=np.float32)
    c0 = np.asarray(inputs["c0"], dtype=np.float32)
    enc = np.ascontiguousarray(np.asarray(inputs["encoder_outputs"], np.float32))
    mask = np.ascontiguousarray(np.asarray(inputs["encoder_mask"]).astype(np.uint8))
    embed = np.ascontiguousarray(np.asarray(inputs["embed"], np.float32))
    wlog = np.asarray(inputs["W_logit"], np.float32)
    blog = np.asarray(inputs["b_logit"], np.float32)
    b0 = (np.asarray(inputs["b_ih0"], np.float32)
          + np.asarray(inputs["b_hh0"], np.float32))
    b1 = (np.asarray(inputs["b_ih1"], np.float32)
          + np.asarray(inputs["b_hh1"], np.float32))
    h0T = np.ascontiguousarray(h0.transpose(0, 2, 1))
    c0T = np.ascontiguousarray(c0.transpose(0, 2, 1))
    Vc = V // N_CORES

    if T not in _BUILD_CACHE:
        _BUILD_CACHE[T] = build_decoder_nc(T)
    nc = _BUILD_CACHE[T]

    common = {
        "trg_seq": trg_seq, "h0T": h0T, "c0T": c0T, "embed": embed,
        "W_enc": np.ascontiguousarray(np.asarray(inputs["W_enc"], np.float32)),
        "b_enc": np.asarray(inputs["b_enc"], np.float32),
        "W_ih0": np.ascontiguousarray(np.asarray(inputs["W_ih0"], np.float32)),
        "W_hh0": np.ascontiguousarray(np.asarray(inputs["W_hh0"], np.float32)),
        "b0": b0,
        "W_ih1": np.ascontiguousarray(np.asarray(inputs["W_ih1"], np.float32)),
        "W_hh1": np.ascontiguousarray(np.asarray(inputs["W_hh1"], np.float32)),
        "b1": b1,
    }
    in_maps = []
    for c in range(N_CORES):
        bs = slice(4 * c, 4 * (c + 1))
        vs = slice(Vc * c, Vc * (c + 1))
        m = dict(common)
        m["enc_loc"] = np.ascontiguousarray(enc[bs])
        m["mask_loc"] = np.ascontiguousarray(mask[bs])
        m["W_logit_loc"] = np.ascontiguousarray(wlog[vs])
        m["b_logit_loc"] = np.ascontiguousarray(blog[vs])
        in_maps.append(m)

    res = run_bass_kernel_spmd(nc, in_maps, core_ids=list(range(N_CORES)))
    LAST_RESULT = res
    out = np.concatenate([res.results[c]["out"] for c in range(N_CORES)], axis=2)
    return out
